# revision 71
# baseline (speedup 1.0000x reference)
"""PASA group-softmax high-pass downsample kernel for 8 Trainium2 NeuronCores.

Reference computation (n=4, c=64, h=w=128, G=2 groups, K=3, stride 2):
  xp     = reflect_pad(x, 1)
  sigma  = conv3x3(xp, conv_w)                    # [n, 18, h, w]
  sigma  = sigma * bn_scale + bn_shift            # BN (inference)
  sigma  = softmax(sigma, axis=1)                 # over all 18 channels
  sigma  = onehot(center) - sigma                 # high-pass
  out[n,g,c,i,j] = sum_k patches[n,g,c,k,i,j] * sigma[n,g,k,i,j]
  return out[:, :, ::2, ::2]                      # [4, 64, 64, 64]

Key optimizations over the v1 kernel (44.9us -> 36.4us measured):
  - PE warm-up needs no DMA (memset lhsT) and is issued before any
    dma_start, so it ramps the PE clock from prelude-end (~7us) instead of
    after the first input lands; filler matmuls bridge the softmax-chain
    gap so the apply matmuls never see the ~1.6x idle-declock penalty.
  - x load split across the 3 DMA rings proportional to their measured
    rates (scalar ~180 GB/s, gpsimd ~60-120, sync ~50-65), scalar's chunk
    in two descriptors, and the conv reordered ch0-quadrants-first so half
    the conv streams before the last x rows land.
  - fp32 xcen input dropped (center comes from the bf16 slab); y output is
    bf16 (host converts), halving the output-store tail.
  - Denominator replicated in ONE bf16 matmul (ones-block lhsT -> D on all
    128 partitions), replacing v1's fp32 LOW_HIGH rbig18 matmul; 1-pass
    reciprocal_approx_fast (tolerance allows it).
  - Apply: ebig tiles pre-issued on a 3-deep PSUM rotation (2-slot pool +
    the recycled d32 bank pair), so each DVE multiply waits only on the
    previous one: the 7 DVE multiplies run back-to-back at ~1.13us. Taps
    2 and 5 run on GpSimd from an ACT-copied SBUF image of ebig (GpSimd
    cannot read PSUM), with dedicated prod tiles + deferred accumulate
    matmuls so the slower GpSimd chain never blocks the DVE stream or the
    in-order PE queue. Tap sums accumulate on the PE (identity matmuls)
    into two per-chunk PSUM tiles (ch0 combine overlaps ch1 accumulate).

Per-core device layout:
  core = (image n, h-half); partitions p<64: channel p, sub-half A (padded
  rows r0..r0+32); p>=64: channel p-64, sub-half B (rows r0+32..r0+64).
  conv -> PSUM sigma [128, 512]: col-group q=(half, chunk) holds sigma
  channels at partitions 32q..32q+18 (rows 18..31 zero weights), 512
  positions each (chunk = 8 output rows x 64 cols).
  E = exp(sigma + bn_shift) (ACT, bias per partition); D via ones-block
  matmul -> [128, 512]; rb = 1/D (DVE); F = E*rb (bf16).
  apply: per tap k: ebig_k = esel_k @ F (PE, [128, 1024] channel layout);
  prod_k = patch_k * ebig_k (DVE/GpSimd alternating, bf16); bf16 add tree;
  y = xc - acc (scalar_tensor_tensor), 2 output DMA rings.
"""

import os
import ml_dtypes
import numpy as np

import concourse.bass as bass
import concourse.tile as tile
from concourse import bacc, mybir
from concourse.bass_utils import run_bass_kernel_spmd

F32 = mybir.dt.float32
BF16 = mybir.dt.bfloat16

N, C, H, W = 4, 64, 128, 128
G, K = 2, 3
K2 = K * K
EPS = 1e-5
NCORES = 8
HO, WO = H // 2, W // 2            # 64, 64 output spatial
ROWS_PER_CORE = HO // 2            # 32 output rows per core (half image)
ROWS_SUB = ROWS_PER_CORE // 2      # 16 output rows per sub-half (A/B)
SLAB_R = 2 * ROWS_SUB + 1          # 33 padded rows per sub-half
SLAB_J = 65                        # deinterleaved: 65 even + 65 odd cols
POS_SUB = ROWS_SUB * WO            # 1024 positions per sub-half
CHUNK_ROWS = ROWS_SUB // 2         # 8 output rows per psum chunk
CHUNK = CHUNK_ROWS * WO            # 512 positions per chunk
NWARM = 52

_compiled = None


def _build_program():
    """Build the single SPMD Bass program (same for all 8 cores)."""
    nc = bacc.Bacc(
        "TRN2", target_bir_lowering=False, debug=False, num_devices=NCORES
    )

    xab = nc.dram_tensor("xab", [128, SLAB_R, 2, SLAB_J], BF16,
                         kind="ExternalInput")
    wts = nc.dram_tensor("wts", [128, K2, 32], BF16, kind="ExternalInput")
    bias = nc.dram_tensor("bias", [128, 1], F32, kind="ExternalInput")
    ones32 = nc.dram_tensor("ones32", [128, 128], BF16, kind="ExternalInput")
    ident = nc.dram_tensor("ident", [128, 128], BF16, kind="ExternalInput")
    esel = nc.dram_tensor("esel", [128, 2 * K2, 128], BF16,
                          kind="ExternalInput")
    y = nc.dram_tensor("y", [128, ROWS_SUB, WO], BF16, kind="ExternalOutput")
    warm_out = nc.dram_tensor("warm_out", [1, 2], F32, kind="ExternalOutput")

    with tile.TileContext(nc) as tc:
        with (
            tc.tile_pool(name="singles", bufs=1) as singles,
            tc.tile_pool(name="psum", bufs=1, space="PSUM") as psum,
            tc.tile_pool(name="ebig", bufs=2, space="PSUM") as ebig_pool,
            tc.tile_pool(name="work", bufs=3) as work,
        ):
            # PE warm-up FIRST: junk matmuls on a memset tile ramp the PE
            # clock from t~0. Must be issued before any dma_start on the
            # gpsimd/tensor queues: dma_start instructions block until the
            # DMA subsystem comes up (~8us), and anything ordered after them
            # inherits that stall.
            wtile = work.tile([128, 128], BF16, tag="wtile")
            nc.gpsimd.memset(wtile[:], 0.5)
            warm_in = work.tile([1, 1], F32, tag="warm_in")
            nc.gpsimd.memset(warm_in[:], 0.25)
            warm_e = work.tile([1, 1], F32, tag="warm")
            nc.scalar.activation(warm_e[:], warm_in[:],
                                 mybir.ActivationFunctionType.Exp)
            warm_ps = psum.tile([128, 128], F32, tag="d32",
                                padded_shape=[128, CHUNK])
            for i in range(NWARM):
                nc.tensor.matmul(warm_ps[:], wtile[:], wtile[:],
                                 start=(i == 0), stop=(i == NWARM - 1),
                                 skip_group_check=True)
            warm_sb = work.tile([1, 2], F32, tag="warm_sb")
            nc.vector.tensor_copy(warm_sb[:], warm_ps[0:1, 0:2])

            # ---- loads: x split over the 3 DMA rings; esel after x ----
            # sync: wts+bias then x; gpsimd: x then ones32/ident/esel half;
            # scalar: x then esel half.
            x_sb = singles.tile([128, 2 * SLAB_R, SLAB_J], BF16)
            esel_sb = singles.tile([128, 2 * K2, 128], BF16)
            ones_sb = singles.tile([128, 128], BF16)
            ident_sb = singles.tile([128, 128], BF16)
            w_sb = singles.tile([128, K2, 32], BF16)
            bias_sb = singles.tile([128, 1], F32)
            xv = x_sb[:].rearrange("p (r e) j -> p r e j", e=2)
            nc.sync.dma_start(w_sb[:], wts.ap())
            nc.sync.dma_start(bias_sb[:], bias.ap())
            # ring speeds differ (~180 GB/s scalar, ~120 gpsimd, ~60 sync):
            # split x accordingly; outputs go on the fast rings later.
            # scalar's chunk is split so the ch0 conv (rows 0..16) can start
            # before scalar's later rows land; rows 18-22 even-first because
            # the dy=0 ch1 taps need only even slab rows. (Finer parity
            # splits of the other chunks made the Tile scheduler interleave
            # the clock-keeper fillers into the conv — measured slower.)
            nc.scalar.dma_start(xv[:, 0:17], xab.ap()[:, 0:17])
            nc.scalar.dma_start(xv[:, 18:23:2], xab.ap()[:, 18:23:2])
            nc.scalar.dma_start(xv[:, 17:24:2], xab.ap()[:, 17:24:2])
            nc.gpsimd.dma_start(xv[:, 24:28], xab.ap()[:, 24:28])
            nc.sync.dma_start(xv[:, 28:SLAB_R], xab.ap()[:, 28:SLAB_R])
            nc.gpsimd.dma_start(ones_sb[:], ones32.ap())
            nc.gpsimd.dma_start(ident_sb[:], ident.ap())
            nc.gpsimd.dma_start(esel_sb[:, 0:9], esel.ap()[:, 0:9])
            nc.sync.dma_start(esel_sb[:, 9:], esel.ap()[:, 9:])
            nc.sync.dma_start(warm_out.ap(), warm_sb[:])

            # ---- conv: 9 taps x 4 col-groups into one PSUM bank ----
            # ch0 quadrants (slab rows 0..16) run before ch1 (rows 16..32),
            # so the first half of the conv streams while the later x rows
            # are still arriving
            sigma_ps = psum.tile([128, CHUNK], F32, tag="sigma")
            for ch in range(2):
                for k in range(K2):
                    dy, dx = k // K, k % K
                    eo, j0 = dx & 1, dx >> 1
                    for h in range(2):
                        q = 2 * h + ch
                        p0 = 64 * h
                        fr = 2 * (2 * CHUNK_ROWS * ch + dy) + eo
                        rhs = x_sb[
                            p0 : p0 + 64,
                            fr : fr + 4 * (CHUNK_ROWS - 1) + 1 : 4,
                            j0 : j0 + WO,
                        ]
                        nc.tensor.matmul(
                            sigma_ps[32 * q : 32 * q + 32, :],
                            w_sb[p0 : p0 + 64, k, :],
                            rhs,
                            start=(k == 0),
                            stop=(k == K2 - 1),
                            tile_position=(p0, 32 * q),
                            skip_group_check=True,
                        )

            # PE filler matmuls keep the clock ramped while the PE waits on
            # the ACT/DVE softmax chain (idle gaps derate the PE clock to
            # ~1.6x slower for the following matmuls). They accumulate junk
            # into the acc bank, which the real accumulation later resets.
            # acc is TWO tiles (one per chunk): dependency tracking is
            # tile-granular, so the ch0 combine (a read) must not serialize
            # against the ch1 accumulate-stop (a write).
            # acc ch0 reuses the sigma bank (free after exp); acc ch1 has
            # its own bank
            acc0 = psum.tile([128, CHUNK], F32, tag="sigma")
            acc1 = psum.tile([128, CHUNK], F32, tag="acc1")
            acc_chs = [acc0, acc1]

            def fillers(n):
                for _ in range(n):
                    nc.tensor.matmul(acc_chs[1][:, 0:128], wtile[:],
                                     wtile[:], start=True, stop=True,
                                     skip_group_check=True)

            fillers(6)

            # ---- E = exp(sigma + bn_shift), in bf16 ----
            e_sb = singles.tile([128, CHUNK], BF16)
            nc.scalar.activation(
                e_sb[:], sigma_ps[:], mybir.ActivationFunctionType.Exp,
                bias=bias_sb[:], scale=1.0,
            )

            # ---- D replicated on all partitions in one bf16 matmul ----
            # (2-bank tile: the bank pair is recycled as the third ebig
            # slot once the reciprocal has consumed D)
            d32_ps = psum.tile([128, POS_SUB], F32, tag="d32")
            nc.tensor.matmul(d32_ps[:, 0:CHUNK], ones_sb[:], e_sb[:])
            fillers(18)
            rb_sb = singles.tile([128, CHUNK], F32)
            nc.vector.reciprocal_approx_fast(rb_sb[:], d32_ps[:, 0:CHUNK])
            f_sb = singles.tile([128, CHUNK], BF16)
            nc.vector.tensor_mul(f_sb[:], e_sb[:], rb_sb[:])

            # ---- apply: prod_k = patch_k * Fbig_k, alternating DVE/GpSimd --
            def patch_view(dy, dx, rows=slice(0, ROWS_SUB)):
                eo, j0 = dx & 1, dx >> 1
                r0, r1 = rows.start, rows.stop
                fr = 2 * (dy + 2 * r0) + eo
                return x_sb[:, fr : fr + 4 * (r1 - r0 - 1) + 1 : 4,
                            j0 : j0 + WO]

            # All 9 per-tap multiplies run on DVE (GpSimd cannot read PSUM
            # and is ~2x slower + contends with DVE on SBUF); the tap SUM
            # runs on the PE as identity-accumulate matmuls. ebig tiles are
            # pre-issued two taps ahead (bufs=2) so each DVE multiply only
            # waits on the previous multiply, never on the PE.
            def make_ebig(k):
                if k % 3 == 2:
                    # third slot: the d32 bank pair, free after the recip
                    t = psum.tile([128, POS_SUB], F32, name=f"ebig{k}",
                                  tag="d32")
                else:
                    t = ebig_pool.tile([128, POS_SUB], F32,
                                       name=f"ebig{k}", tag="ebig")
                for ch in range(2):
                    nc.tensor.matmul(
                        t[:, CHUNK * ch : CHUNK * (ch + 1)],
                        esel_sb[:, 2 * k + ch, :],
                        f_sb[:],
                    )
                return t

            def acc_prod(k, prod):
                pflat = prod[:].rearrange("p r c -> p (r c)")
                for ch in range(2):
                    nc.tensor.matmul(
                        acc_chs[ch][:],
                        ident_sb[:],
                        pflat[:, CHUNK * ch : CHUNK * (ch + 1)],
                        start=(k == 0),
                        stop=(k == K2 - 1),
                        skip_group_check=True,
                    )

            # Taps in GPS_TAPS run on GpSimd from an ACT-copied SBUF image
            # of ebig (GpSimd cannot read PSUM); the rest multiply on DVE
            # straight from PSUM.
            # GPS_TAPS multiply on GpSimd from an ACT-copied bf16 SBUF image
            # (GpSimd cannot read PSUM); the rest multiply on DVE straight
            # from PSUM. Copy-feeding DVE taps was measured SLOWER (both
            # operands on the SBUF port contend; the PSUM operand rides a
            # separate port), so CPY_TAPS stays empty.
            GPS_TAPS = (2, 5)
            CPY_TAPS = ()
            ebigs = {0: make_ebig(0), 1: make_ebig(1), 2: make_ebig(2)}
            prods = []
            for k in range(K2):
                dy, dx = k // K, k % K
                e3 = ebigs[k][:].rearrange("p (r c) -> p r c", r=ROWS_SUB)
                if k in GPS_TAPS or k in CPY_TAPS:
                    ecp = work.tile([128, POS_SUB], BF16, tag="ecp")
                    nc.scalar.activation(
                        ecp[:], ebigs[k][:],
                        mybir.ActivationFunctionType.Copy)
                    ecp3 = ecp[:].rearrange("p (r c) -> p r c", r=ROWS_SUB)
                if k in GPS_TAPS:
                    # dedicated (non-rotating) prod tile + deferred acc so
                    # the slower GpSimd chain never blocks the DVE stream
                    # or the in-order PE accumulate queue
                    prod = singles.tile([128, ROWS_SUB, WO], BF16,
                                        name=f"prodg{k}")
                    nc.gpsimd.tensor_mul(prod[:], patch_view(dy, dx), ecp3)
                else:
                    prod = work.tile([128, ROWS_SUB, WO], BF16,
                                     name=f"prod{k}", tag="prod")
                    if k in CPY_TAPS:
                        nc.vector.tensor_mul(prod[:], patch_view(dy, dx),
                                             ecp3)
                    elif k < K2 - 1:
                        nc.vector.tensor_mul(prod[:], patch_view(dy, dx),
                                             e3)
                prods.append(prod)
                if k + 3 < K2:
                    ebigs[k + 3] = make_ebig(k + 3)
                if k < K2 - 1 and k not in GPS_TAPS:
                    acc_prod(k, prod)

            # ---- tail: mul8 split per chunk so each chunk's acc-stop,
            # combine, and store start as soon as its half-product lands ----
            for k in GPS_TAPS:
                acc_prod(k, prods[k])
            e3last = ebigs[K2 - 1][:].rearrange("p (r c) -> p r c",
                                                r=ROWS_SUB)
            y_sb = work.tile([128, ROWS_SUB, WO], BF16, tag="y")
            pf = prods[K2 - 1][:].rearrange("p r c -> p (r c)")
            ring_splits = [
                [(nc.scalar, slice(0, 8))],
                [(nc.gpsimd, slice(8, 12)), (nc.sync, slice(12, 16))],
            ]
            for ch in range(2):
                rr = slice(CHUNK_ROWS * ch, CHUNK_ROWS * (ch + 1))
                nc.vector.tensor_mul(prods[K2 - 1][:, rr],
                                     patch_view(2, 2, rows=rr),
                                     e3last[:, rr])
                nc.tensor.matmul(
                    acc_chs[ch][:],
                    ident_sb[:],
                    pf[:, CHUNK * ch : CHUNK * (ch + 1)],
                    start=False,
                    stop=True,
                    skip_group_check=True,
                )
                acc3 = acc_chs[ch][:].rearrange("p (r c) -> p r c",
                                                r=CHUNK_ROWS)
                xc = patch_view(1, 1, rows=rr)
                nc.vector.tensor_sub(y_sb[:, rr], xc, acc3)
                for eng, rs in ring_splits[ch]:
                    eng.dma_start(y.ap()[:, rs], y_sb[:, rs])

    nc.compile()
    return nc


def _host_inputs(x, conv_w, gamma, beta, running_mean, running_var):
    """Prepare per-core input dicts (sharding + BN folding + reflect pad)."""
    scale = gamma / np.sqrt(running_var + EPS)
    shift = beta - running_mean * scale

    # weights: lhsT layout [tap, c, o] scaled by BN, padded to 32 outs, dup'd
    w_scaled = conv_w * scale[:, None, None, None]           # [18, 64, 3, 3]
    wl = np.transpose(w_scaled, (2, 3, 1, 0)).reshape(K2, C, G * K2)
    wl32 = np.zeros((K2, C, 32), np.float32)
    wl32[:, :, : G * K2] = wl
    wts = np.ascontiguousarray(
        np.concatenate([wl32, wl32], axis=1).transpose(1, 0, 2)
    ).astype(ml_dtypes.bfloat16)
    # -> [128, 9, 32]

    bias = np.zeros((128, 1), np.float32)
    for q in range(4):
        bias[32 * q : 32 * q + G * K2, 0] = shift

    # ones-block: d32[p, pos] = sum of e rows in p's quadrant (lhsT layout)
    ones32 = np.zeros((128, 128), np.float32)
    for q in range(4):
        ones32[32 * q : 32 * q + G * K2, 32 * q : 32 * q + 32] = 1.0
    ones32 = ones32.astype(ml_dtypes.bfloat16)

    ident_m = np.eye(128, dtype=np.float32).astype(ml_dtypes.bfloat16)

    # esel[:, 2k+c, :]: lhsT mapping F rows -> channel-layout partitions for
    # tap k, chunk c.
    esel = np.zeros((128, 2 * K2, 128), np.float32)
    for k in range(K2):
        for c in range(2):
            for j in range(128):
                h, g = j // 64, (j % 64) // 32
                esel[32 * (2 * h + c) + g * K2 + k, 2 * k + c, j] = 1.0
    esel = esel.astype(ml_dtypes.bfloat16)

    xpad = np.pad(x, ((0, 0), (0, 0), (1, 1), (1, 1)), mode="reflect")

    in_maps = []
    for core in range(NCORES):
        n, h = core // 2, core % 2
        r0 = 64 * h
        slab_a = xpad[n, :, r0 : r0 + SLAB_R, :]
        slab_b = xpad[n, :, r0 + SLAB_R - 1 : r0 + 2 * SLAB_R - 1, :]
        xab = np.concatenate([slab_a, slab_b], axis=0)       # [128, 33, 130]
        xde = np.stack([xab[:, :, 0::2], xab[:, :, 1::2]], axis=2)
        xde = np.ascontiguousarray(xde, np.float32)          # [128,33,2,65]
        in_maps.append(
            {"xab": xde.astype(ml_dtypes.bfloat16), "wts": wts,
             "bias": bias, "ones32": ones32, "ident": ident_m, "esel": esel}
        )
    return in_maps


def _gather_output(results):
    out = np.empty((N, C, HO, WO), np.float32)
    for core, res in enumerate(results):
        n, h = core // 2, core % 2
        ycore = np.asarray(res["y"]).astype(np.float32)
        ycore = ycore.reshape(2, C, ROWS_SUB, WO)
        out[n, :, 32 * h : 32 * h + ROWS_SUB, :] = ycore[0]
        out[n, :, 32 * h + ROWS_SUB : 32 * h + 2 * ROWS_SUB, :] = ycore[1]
    return out


def _ensure_ntff_hook():
    """Install the axon NTFF profile hook if the image's antenv lacks it."""
    try:
        from antenv import axon_hooks  # noqa: F401
        return
    except ImportError:
        pass
    try:
        import sys
        import types

        import antenv
        from trn_agent_boot.trn_boot import _ntff_profile_via_ctypes

        hook = _ntff_profile_via_ctypes("/opt/axon/libaxon_pjrt.so")
        mod = types.ModuleType("antenv.axon_hooks")
        state = {"hook": hook}
        mod.get_axon_ntff_profile_hook = lambda: state["hook"]
        mod.set_axon_ntff_profile_hook = lambda h: state.update(hook=h)
        sys.modules["antenv.axon_hooks"] = mod
        antenv.axon_hooks = mod
    except Exception:
        pass


def kernel(x, conv_w, gamma, beta, running_mean, running_var):
    global _compiled
    x = np.asarray(x, np.float32)
    conv_w = np.asarray(conv_w, np.float32)
    gamma = np.asarray(gamma, np.float32)
    beta = np.asarray(beta, np.float32)
    running_mean = np.asarray(running_mean, np.float32)
    running_var = np.asarray(running_var, np.float32)

    if _compiled is None:
        _compiled = _build_program()
    nc = _compiled

    in_maps = _host_inputs(x, conv_w, gamma, beta, running_mean, running_var)
    trace = bool(int(os.environ.get("PASA_TRACE", "0")))
    if trace:
        _ensure_ntff_hook()
    res = run_bass_kernel_spmd(
        nc, in_maps, core_ids=list(range(NCORES)), trace=trace
    )
    kernel.last_results = res
    return _gather_output(res.results)


if __name__ == "__main__":
    # quick CoreSim check of core 0 against a numpy re-implementation
    from concourse.bass_interp import CoreSim

    rng = np.random.default_rng(0)
    x = rng.standard_normal((N, C, H, W)).astype(np.float32)
    conv_w = (rng.standard_normal((G * K2, C, K, K)).astype(np.float32)
              * np.sqrt(2.0 / (G * K2 * K * K)))
    gamma = rng.uniform(0.5, 1.5, G * K2).astype(np.float32)
    beta = (rng.standard_normal(G * K2) * 0.1).astype(np.float32)
    rmean = (rng.standard_normal(G * K2) * 0.1).astype(np.float32)
    rvar = rng.uniform(0.5, 1.5, G * K2).astype(np.float32)

    nc = _build_program()
    in_maps = _host_inputs(x, conv_w, gamma, beta, rmean, rvar)
    sim = CoreSim(nc)
    for kk, v in in_maps[0].items():
        sim.tensor(kk)[:] = v
    sim.simulate(check_with_hw=False)
    ysim = (np.array(sim.tensor("y")).astype(np.float32)
            .reshape(2, C, ROWS_SUB, WO))

    # numpy reference for core 0 region (image 0, output rows 0..32)
    scale = gamma / np.sqrt(rvar + EPS)
    shift = beta - rmean * scale
    xpad = np.pad(x[0], ((0, 0), (1, 1), (1, 1)), mode="reflect")
    sig = np.zeros((G * K2, 32, WO), np.float32)
    for o in range(G * K2):
        for dy in range(K):
            for dx in range(K):
                sig[o] += np.einsum(
                    "crw->rw",
                    conv_w[o, :, dy, dx][:, None, None]
                    * xpad[:, dy : dy + 64 : 2, dx : dx + 128 : 2],
                )
    sig = sig * scale[:, None, None] + shift[:, None, None]
    e = np.exp(sig)
    r = 1.0 / e.sum(0)
    acc = np.zeros((C, 32, WO), np.float32)
    for g in range(G):
        for k in range(K2):
            dy, dx = k // K, k % K
            acc[32 * g : 32 * g + 32] += (
                xpad[32 * g : 32 * g + 32, dy : dy + 64 : 2, dx : dx + 128 : 2]
                * e[g * K2 + k][None]
                * r[None]
            )
    ref = (xpad[:, 1:65:2, 1:129:2] - acc).astype(np.float32)

    got = np.concatenate([ysim[0], ysim[1]], axis=1)
    err = np.abs(got - ref).max() / np.abs(ref).max()
    print("sim rel err:", err)
